# revision 1
# baseline (speedup 1.0000x reference)
"""Multi-head SAGE layer (mean aggregation) as a Bass/Tile kernel on 8 trn2 cores.

Math: out = mean_h( h @ W_self[h] + segmean(h[src] by dst) @ W_neigh[h] + b[h] )
    = h @ mean_h(W_self) + segmean(h[src] by dst) @ mean_h(W_neigh) + mean_h(b)
(mean over heads commutes with the linear layers).

Sharding: nodes (and their incident edges, keyed by dst) are split across the
8 cores, 12500 nodes each; h is replicated per-core (bf16 copy) as the gather
table.  Edges are grouped by (src-range bucket, 128-node dst block), sorted
by src, and padded per group to R[b,k] = T[b,k]*128 slots (T = max count over
the 8 cores so the SPMD schedule is shared; pad slots gather row 0 of the
bucket and are masked out of the segment-sum by rseg = -1).  Each bucket's
slot stream is cut into fixed 1024-slot gather windows (the dma_gather ucode
caps one call at 1024 indices); dst-block groups freely straddle window
boundaries — only 128-slot sub-tile alignment is preserved.  One dma_gather
call fetches a whole window; the SWDGE descriptor ring is enlarged to 4096
descriptors per queue so several calls stay in flight per queue.

The segment-sum runs on the tensor engine in bf16, directly in transposed
orientation: per 128-edge sub-tile, matmul(lhsT=E, rhs=msel) accumulates
S^T[feat, node] in PSUM, where msel[e, n] = (dst_local[e] == n) is built for
ALL of a block's sub-tiles with a single wide vector is_equal (stride-0
broadcast AP).  S^T feeds the W_neigh matmul without any transpose; the
1/deg normalization is applied to the matmul OUTPUT (diagonal scaling
commutes), and the self term + bias accumulate in a second PSUM tile.

All graph-structure preprocessing (edge partition/sort/padding, degree
counts, layout transposes, bf16 casts) happens on the host; all matrix math
on the features/weights happens on-device.
"""

import sys

import ml_dtypes
import numpy as np

for _p in ("/opt/trn_rl_repo",):
    if _p not in sys.path:
        sys.path.insert(0, _p)

BF16 = ml_dtypes.bfloat16

N_NODES = 100000
N_EDGES = 1600000
D = 128
H = 4
N_CORES = 8
P = 128
NPC = N_NODES // N_CORES          # nodes per core
NB = (NPC + P - 1) // P           # 128-node blocks per core
NPAD = NB * P                     # padded nodes per core
NBUCKETS = 4
BUCKET_SZ = -(-N_NODES // NBUCKETS)   # src rows per gather bucket (int16 range)
WSLOTS = 1024                     # gather-window slots (ucode per-call max)
RING = 65536                      # dynamic_dma_scratch_size -> 4096 desc/queue


def _preprocess(src, dst):
    """Partition edges by dst owner core, group by (128-node dst block, src
    bucket), sort by src within each group, pad each group to R[b,k] slots.

    Returns (per_core, sched) where per_core[c]:
      idx16 [128, sumW16] int16  bucket-local gather indices, wrapped-16
                                 layout, one contiguous stream per bucket;
                                 pad = 0
      rseg  [128, sumT] bf16     dst id local to the block in [0,128),
                                 pad = -1; column order (b, k, t)
      deg   [128, NB]   int32    in-degree per node, column b = block b
    """
    deg_full = np.bincount(dst, minlength=N_NODES).astype(np.int32)
    counts = np.zeros((N_CORES, NB, NBUCKETS), np.int64)
    blocks = []
    for c in range(N_CORES):
        lo = c * NPC
        m = (dst >= lo) & (dst < lo + NPC)
        s_c = src[m].astype(np.int64)
        d_c = (dst[m] - lo).astype(np.int64)
        blk = d_c >> 7
        bkt = s_c // BUCKET_SZ
        order = np.lexsort((s_c, bkt, blk))
        s_c, d_c, blk, bkt = s_c[order], d_c[order], blk[order], bkt[order]
        key = (blk * NBUCKETS + bkt)
        bc = np.bincount(key, minlength=NB * NBUCKETS)
        counts[c] = bc.reshape(NB, NBUCKETS)
        off = np.zeros(NB * NBUCKETS + 1, np.int64)
        np.cumsum(bc, out=off[1:])
        blocks.append((s_c, d_c, off))

    cmax = counts.max(axis=0)                            # [NB, NBUCKETS]
    T = (-(-np.maximum(1, cmax) // P)).astype(np.int64)  # matmul sub-tiles
    R = T * P                                            # gather slots

    # rseg/msel column offsets, (b, k, t) order (block-major)
    col_off = np.zeros((NB, NBUCKETS), np.int64)
    acc_t = 0
    for b in range(NB):
        for k in range(NBUCKETS):
            col_off[b, k] = acc_t
            acc_t += T[b, k]
    sumT = int(acc_t)
    nmm = T.sum(axis=1).astype(np.int64)                 # matmuls per block
    nmmax = int(nmm.max())

    # per-bucket slot streams: group (b,k) starts at gstart[b,k].  Streams
    # are padded to a common nwin*WSLOTS length so gather windows can be
    # issued in strict queue round-robin (Tile locks each of its 8 DMASW
    # semaphore lanes — assigned to SWDGE instructions in emission order —
    # to a single queue, so emissions must cycle queues 0..3).
    gstart = np.zeros((NB, NBUCKETS), np.int64)
    bucket_len = np.zeros(NBUCKETS, np.int64)
    for k in range(NBUCKETS):
        pos = 0
        for b in range(NB):
            gstart[b, k] = pos
            pos += int(R[b, k])
        bucket_len[k] = pos
    nwin = int(max(-(-int(bucket_len[k]) // WSLOTS)
                   for k in range(NBUCKETS)))
    wcols = WSLOTS // 16
    bucket_ioff = np.array([k * nwin * wcols for k in range(NBUCKETS)],
                           np.int64)
    sumW16 = NBUCKETS * nwin * wcols

    per_core = []
    for c in range(N_CORES):
        s_c, d_c, off = blocks[c]
        idx_cols = np.zeros((16, sumW16), np.int16)
        rseg_flat = np.full(sumT * P, -1.0, np.float32)
        for b in range(NB):
            for k in range(NBUCKETS):
                n = int(counts[c, b, k])
                if n == 0:
                    continue
                o = int(off[b * NBUCKETS + k])
                flat = np.zeros(int(R[b, k]), np.int64)
                flat[:n] = s_c[o:o + n] - k * BUCKET_SZ
                w0 = int(bucket_ioff[k]) + int(gstart[b, k]) // 16
                idx_cols[:, w0:w0 + int(R[b, k]) // 16] = flat.reshape(
                    -1, 16).T
                base = int(col_off[b, k]) * P
                rseg_flat[base:base + n] = (d_c[o:o + n] - b * P).astype(
                    np.float32)
        idx16 = np.tile(np.ascontiguousarray(idx_cols), (8, 1))
        rseg_t = np.ascontiguousarray(
            rseg_flat.reshape(sumT, P).T).astype(BF16)
        degc = np.zeros(NPAD, np.int32)
        degc[:NPC] = deg_full[c * NPC:(c + 1) * NPC]
        deg_t = np.ascontiguousarray(degc.reshape(NB, P).T)
        per_core.append({"idx16": idx16, "rseg": rseg_t, "deg": deg_t})

    sched = {
        "T": T, "R": R, "col_off": col_off, "gstart": gstart,
        "bucket_len": bucket_len, "bucket_ioff": bucket_ioff,
        "nwin": nwin, "sumT": sumT, "sumW16": sumW16,
        "nmm": nmm, "nmmax": nmmax,
    }
    return per_core, sched


def build_program(sched, n_nodes=N_NODES, nb=NB, npad=NPAD):
    """Trace + compile the SPMD Bass program for the given group schedule."""
    from contextlib import ExitStack

    from concourse import bacc, mybir, tile

    f32 = mybir.dt.float32
    bf16 = mybir.dt.bfloat16
    i32 = mybir.dt.int32
    i16 = mybir.dt.int16
    AL = mybir.AluOpType

    T = sched["T"]
    R = sched["R"]
    col_off = sched["col_off"]
    gstart = sched["gstart"]
    bucket_len = sched["bucket_len"]
    bucket_ioff = sched["bucket_ioff"]
    nwin = sched["nwin"]
    sumT = sched["sumT"]
    sumW16 = sched["sumW16"]
    nmm = sched["nmm"]
    nmmax = sched["nmmax"]

    nc = bacc.Bacc("TRN2", target_bir_lowering=False, debug=False,
                   num_devices=N_CORES, num_swdge_queues=NBUCKETS,
                   dynamic_dma_scratch_size=RING)
    h_ap = nc.dram_tensor("h16", [n_nodes, D], bf16, kind="ExternalInput").ap()
    hT_ap = nc.dram_tensor("hT16", [P, npad], bf16, kind="ExternalInput").ap()
    idx_ap = nc.dram_tensor("idx16", [P, sumW16], i16,
                            kind="ExternalInput").ap()
    rseg_ap = nc.dram_tensor("rseg", [P, sumT], bf16,
                             kind="ExternalInput").ap()
    deg_ap = nc.dram_tensor("deg", [P, nb], i32, kind="ExternalInput").ap()
    iota_ap = nc.dram_tensor("iotaw", [P, nmmax * P], bf16,
                             kind="ExternalInput").ap()
    ws_ap = nc.dram_tensor("W_self", [H, D, D], f32, kind="ExternalInput").ap()
    wn_ap = nc.dram_tensor("W_neigh", [H, D, D], f32,
                           kind="ExternalInput").ap()
    b_ap = nc.dram_tensor("b", [H, D], f32, kind="ExternalInput").ap()
    out_ap = nc.dram_tensor("out", [npad, D], f32, kind="ExternalOutput").ap()

    bucket_aps = []
    for k in range(NBUCKETS):
        lo = k * BUCKET_SZ
        hi = min(n_nodes, lo + BUCKET_SZ)
        bucket_aps.append(h_ap[lo:hi, :])

    with tile.TileContext(nc) as tc, ExitStack() as ctx:
        const = ctx.enter_context(tc.tile_pool(name="const", bufs=1))
        eps = [ctx.enter_context(tc.tile_pool(name=f"eg{k}", bufs=6))
               for k in range(NBUCKETS)]
        mp = ctx.enter_context(tc.tile_pool(name="msel", bufs=2))
        sp = ctx.enter_context(tc.tile_pool(name="small", bufs=3))
        pseg = ctx.enter_context(tc.tile_pool(name="pseg", bufs=2,
                                              space="PSUM"))
        pout = ctx.enter_context(tc.tile_pool(name="pout", bufs=2,
                                              space="PSUM"))
        pslf = ctx.enter_context(tc.tile_pool(name="pslf", bufs=2,
                                              space="PSUM"))

        # ---- prologue: constants ----
        iotaw = const.tile([P, nmmax * P], bf16, tag="iotaw")
        nc.sync.dma_start(iotaw[:], iota_ap)

        # head-averaged weights: wm = 0.25 * sum_h W[h], cast to bf16
        wmeans = []
        for name, ap in (("ws", ws_ap), ("wn", wn_ap)):
            heads = []
            for hh in range(H):
                t = const.tile([P, P], f32, tag=f"{name}h{hh}")
                nc.sync.dma_start(t[:], ap[hh])
                heads.append(t)
            s01 = const.tile([P, P], f32, tag=f"{name}s01")
            nc.vector.tensor_tensor(s01[:], heads[0][:], heads[1][:],
                                    op=AL.add)
            s23 = const.tile([P, P], f32, tag=f"{name}s23")
            nc.vector.tensor_tensor(s23[:], heads[2][:], heads[3][:],
                                    op=AL.add)
            s = const.tile([P, P], f32, tag=f"{name}sum")
            nc.vector.tensor_tensor(s[:], s01[:], s23[:], op=AL.add)
            wm = const.tile([P, P], bf16, tag=f"{name}m")
            nc.scalar.mul(wm[:], s[:], 1.0 / H)
            wmeans.append(wm)
        wsm, wnm = wmeans

        # bias matmul operands: q[h, m] = 1/H; pself += q.T @ b_sb
        b_sb = const.tile([H, P], f32, tag="bsb")
        nc.sync.dma_start(b_sb[:], b_ap)
        b16 = const.tile([H, P], bf16, tag="b16")
        nc.vector.tensor_copy(b16[:], b_sb[:])
        q16 = const.tile([H, P], bf16, tag="q16")
        nc.vector.memset(q16[:], 1.0 / H)

        # inverse degree: 1 / max(deg, 1)
        degsb = const.tile([P, nb], i32, tag="degsb")
        nc.sync.dma_start(degsb[:], deg_ap)
        degf = const.tile([P, nb], f32, tag="degf")
        nc.vector.tensor_copy(degf[:], degsb[:])
        nc.vector.tensor_scalar_max(degf[:], degf[:], 1.0)
        invd = const.tile([P, nb], f32, tag="invd")
        nc.vector.reciprocal(invd[:], degf[:])

        # edge structure, resident in SBUF
        idx_all = const.tile([P, sumW16], i16, tag="idx_all")
        bchunk = sumW16 // NBUCKETS
        for k in range(NBUCKETS):
            nc.sync.dma_start(idx_all[:, k * bchunk:(k + 1) * bchunk],
                              idx_ap[:, k * bchunk:(k + 1) * bchunk])
        rseg_all = const.tile([P, sumT], bf16, tag="rseg_all")
        nc.sync.dma_start(rseg_all[:], rseg_ap)

        win_tiles = [{} for _ in range(NBUCKETS)]
        rounds = [0]
        prev_gather = [None]

        def issue_round():
            w = rounds[0]
            for k in range(NBUCKETS):
                E = eps[k].tile([P, WSLOTS], bf16, tag=f"E{k}")
                i0 = int(bucket_ioff[k]) + w * (WSLOTS // 16)
                g = nc.gpsimd.dma_gather(
                    E[:].rearrange("p (c d) -> p c d", d=D),
                    bucket_aps[k],
                    idx_all[:, i0:i0 + WSLOTS // 16],
                    WSLOTS,
                    WSLOTS,
                    D,
                    queue_num=k,
                )
                # chain gathers so the Tile scheduler keeps emission order
                # (its 8 DMASW sem lanes are assigned round-robin in schedule
                # order and each lane must stay on one SWDGE queue)
                if prev_gather[0] is not None:
                    from concourse.instruction_name_ordered_set import (
                        InstructionNameOrderedSet)
                    s = InstructionNameOrderedSet()
                    s.add(prev_gather[0])
                    g.ins.add_nosync_dependencies_from(s)
                prev_gather[0] = g.ins.name
                win_tiles[k][w] = E
                if w - 6 in win_tiles[k]:
                    del win_tiles[k][w - 6]
            rounds[0] = w + 1

        for _ in range(min(3, nwin)):
            issue_round()

        # ---- main loop over 128-node dst blocks ----
        for b in range(nb):
            # prefetch: ensure windows covering this block (+1 ahead) issued
            need = max((int(gstart[b, k]) + int(R[b, k]) - P) // WSLOTS
                       for k in range(NBUCKETS))
            while rounds[0] <= min(need + 2, nwin - 1):
                issue_round()

            nb_mm = int(nmm[b])
            c0 = int(col_off[b, 0])

            # wide msel: one is_equal over all of this block's sub-tiles
            msel = mp.tile([P, nmmax * P], bf16, tag="msel")
            nc.vector.tensor_tensor(
                out=msel[:, :nb_mm * P].rearrange("p (c d) -> p c d", d=P),
                in0=rseg_all[:, c0:c0 + nb_mm].to_broadcast([P, nb_mm, P]),
                in1=iotaw[:, :nb_mm * P].rearrange("p (c d) -> p c d", d=P),
                op=AL.is_equal,
            )

            # segment-sum, transposed: psT[feat, node] += E_t.T @ msel_t
            ps = pseg.tile([P, P], f32, tag="seg")
            i = 0
            for k in range(NBUCKETS):
                Tk = int(T[b, k])
                mo = int(col_off[b, k]) - c0
                for t in range(Tk):
                    slot = int(gstart[b, k]) + t * P
                    E = win_tiles[k][slot // WSLOTS]
                    off = slot % WSLOTS
                    nc.tensor.matmul(
                        ps[:],
                        lhsT=E[:, off:off + P],
                        rhs=msel[:, (mo + t) * P:(mo + t + 1) * P],
                        start=(i == 0),
                        stop=(i == nb_mm - 1),
                    )
                    i += 1

            # S^T to SBUF in bf16 (ACT engine)
            psb = sp.tile([P, P], bf16, tag="psb")
            nc.scalar.copy(psb[:], ps[:])

            # neighbor term: po[node, dout] = S.T.T @ wnm  (un-normalized)
            po = pout.tile([P, P], f32, tag="po")
            nc.tensor.matmul(po[:], lhsT=psb[:], rhs=wnm[:],
                             start=True, stop=True)

            # self term + bias: pself[node, dout] = h_blk @ wsm + 1/H sum b
            hTt = sp.tile([P, P], bf16, tag="hTt")
            nc.sync.dma_start(hTt[:], hT_ap[:, b * P:(b + 1) * P])
            pf = pslf.tile([P, P], f32, tag="pself")
            nc.tensor.matmul(pf[:], lhsT=hTt[:], rhs=wsm[:],
                             start=True, stop=False)
            nc.tensor.matmul(pf[:], lhsT=q16[:], rhs=b16[:],
                             start=False, stop=True)

            # out = po * invd[node] + pself
            ot = sp.tile([P, P], f32, tag="ot")
            nc.vector.tensor_scalar(out=ot[:], in0=po[:],
                                    scalar1=invd[:, b:b + 1], scalar2=None,
                                    op0=AL.mult)
            ob = sp.tile([P, P], f32, tag="ob")
            nc.vector.tensor_tensor(out=ob[:], in0=ot[:], in1=pf[:],
                                    op=AL.add)
            nc.sync.dma_start(out_ap[b * P:(b + 1) * P, :], ob[:])

    nc.compile()
    return nc


_CACHE = {}


def kernel(h, src, dst, W_self, W_neigh, b):
    return run(h, src, dst, W_self, W_neigh, b)[0]


def run(h, src, dst, W_self, W_neigh, b, trace=False, **kw):
    from concourse.bass_utils import run_bass_kernel_spmd

    h = np.ascontiguousarray(np.asarray(h, dtype=np.float32))
    src = np.asarray(src, dtype=np.int32)
    dst = np.asarray(dst, dtype=np.int32)
    W_self = np.ascontiguousarray(np.asarray(W_self, dtype=np.float32))
    W_neigh = np.ascontiguousarray(np.asarray(W_neigh, dtype=np.float32))
    b = np.ascontiguousarray(np.asarray(b, dtype=np.float32))

    per_core, sched = _preprocess(src, dst)

    key = (tuple(sched["R"].ravel().tolist()),)
    if key not in _CACHE:
        _CACHE[key] = build_program(sched)
    nc = _CACHE[key]

    nmmax = sched["nmmax"]
    iotaw = np.ascontiguousarray(
        np.tile(np.arange(P, dtype=np.float32), (P, nmmax))).astype(BF16)
    h16 = h.astype(BF16)
    in_maps = []
    for c in range(N_CORES):
        hTc = np.zeros((P, NPAD), np.float32)
        hTc[:, :NPC] = h[c * NPC:(c + 1) * NPC].T
        in_maps.append({
            "h16": h16,
            "hT16": np.ascontiguousarray(hTc).astype(BF16),
            "idx16": per_core[c]["idx16"],
            "rseg": per_core[c]["rseg"],
            "deg": per_core[c]["deg"],
            "iotaw": iotaw,
            "W_self": W_self,
            "W_neigh": W_neigh,
            "b": b,
        })

    res = run_bass_kernel_spmd(nc, in_maps, core_ids=list(range(N_CORES)),
                               trace=trace, **kw)
    out = np.concatenate([res.results[c]["out"][:NPC]
                          for c in range(N_CORES)], axis=0)
    return out, res



# revision 5
# speedup vs baseline: 1.1174x; 1.1174x over previous
"""Multi-head SAGE layer (mean aggregation) as a Bass/Tile kernel on 8 trn2 cores.

Math: out = mean_h( h @ W_self[h] + segmean(h[src] by dst) @ W_neigh[h] + b[h] )
    = h @ mean_h(W_self) + segmean(h[src] by dst) @ mean_h(W_neigh) + mean_h(b)
(mean over heads commutes with the linear layers).

Sharding: nodes (and their incident edges, keyed by dst) are split across the
8 cores, 12500 nodes each; h is replicated per-core (bf16 copy) as the gather
table.  Edges are grouped by (src-range bucket, 128-node dst block), sorted
by src, and padded per group to R[b,k] = T[b,k]*128 slots (T = max count over
the 8 cores so the SPMD schedule is shared; pad slots gather row 0 of the
bucket and are masked out of the segment-sum by rseg = -1).  Each bucket's
slot stream is cut into 4096-slot gather windows (big windows amortize the
~1us fixed Q7 ucode cost per dma_gather call -- the SWDGE descriptor
generation on the GpSimd engine is strictly serialized and paces the whole
kernel); the last window per bucket is exactly the stream remainder, so no
tail slots exist at all.

The segment-sum runs on the tensor engine in bf16, directly in transposed
orientation: per 128-edge sub-tile, matmul(lhsT=E, rhs=msel) accumulates
S^T[feat, node] in PSUM, where msel[e, n] = (dst_local[e] == n) is built for
ALL of a block's sub-tiles with a single wide vector is_equal (stride-0
broadcast AP).  S^T feeds the W_neigh matmul without any transpose; the
1/deg normalization and the self-term addition are fused into one
scalar_tensor_tensor op: out = (po * invd) + pself, written in bf16.

All graph-structure preprocessing (edge partition/sort/padding, degree
counts, layout transposes, bf16 casts) happens on the host; all matrix math
on the features/weights happens on-device.
"""

import sys

import ml_dtypes
import numpy as np

for _p in ("/opt/trn_rl_repo",):
    if _p not in sys.path:
        sys.path.insert(0, _p)

BF16 = ml_dtypes.bfloat16

N_NODES = 100000
N_EDGES = 1600000
D = 128
H = 4
N_CORES = 8
P = 128
NPC = N_NODES // N_CORES          # nodes per core
NB = (NPC + P - 1) // P           # 128-node blocks per core
NPAD = NB * P                     # padded nodes per core
NBUCKETS = 4
BUCKET_SZ = -(-N_NODES // NBUCKETS)   # src rows per gather bucket (int16 range)
WSLOTS = 4096                     # gather-window slots per call
RING = 65536                      # dynamic_dma_scratch_size
HT_CHUNK = 4                      # dst blocks per hT load (bigger descriptors)


def _preprocess(src, dst):
    """Partition edges by dst owner core, group by (128-node dst block, src
    bucket), sort by src within each group, pad each group to R[b,k] slots.

    Returns (per_core, sched) where per_core[c]:
      idx16 [128, sumW16] int16  bucket-local gather indices, wrapped-16
                                 layout, one contiguous stream per bucket;
                                 pad = 0
      rseg  [128, sumT] bf16     dst id local to the block in [0,128),
                                 pad = -1; column order (b, k, t)
      deg   [128, NB]   int32    in-degree per node, column b = block b
    """
    deg_full = np.bincount(dst, minlength=N_NODES).astype(np.int32)
    counts = np.zeros((N_CORES, NB, NBUCKETS), np.int64)
    blocks = []
    for c in range(N_CORES):
        lo = c * NPC
        m = (dst >= lo) & (dst < lo + NPC)
        s_c = src[m].astype(np.int64)
        d_c = (dst[m] - lo).astype(np.int64)
        blk = d_c >> 7
        bkt = s_c // BUCKET_SZ
        order = np.lexsort((s_c, bkt, blk))
        s_c, d_c, blk, bkt = s_c[order], d_c[order], blk[order], bkt[order]
        key = (blk * NBUCKETS + bkt)
        bc = np.bincount(key, minlength=NB * NBUCKETS)
        counts[c] = bc.reshape(NB, NBUCKETS)
        off = np.zeros(NB * NBUCKETS + 1, np.int64)
        np.cumsum(bc, out=off[1:])
        blocks.append((s_c, d_c, off))

    cmax = counts.max(axis=0)                            # [NB, NBUCKETS]
    T = (-(-np.maximum(1, cmax) // P)).astype(np.int64)  # matmul sub-tiles
    R = T * P                                            # gather slots

    # rseg/msel column offsets, (b, k, t) order (block-major)
    col_off = np.zeros((NB, NBUCKETS), np.int64)
    acc_t = 0
    for b in range(NB):
        for k in range(NBUCKETS):
            col_off[b, k] = acc_t
            acc_t += T[b, k]
    sumT = int(acc_t)
    nmm = T.sum(axis=1).astype(np.int64)                 # matmuls per block
    nmmax = int(nmm.max())

    # per-bucket slot streams: group (b,k) starts at gstart[b,k].  Streams
    # are cut into WSLOTS-sized gather windows; the last window per bucket is
    # the exact remainder (all windows share count nwin so calls can be
    # issued in strict queue round-robin -- Tile locks each of its 8 DMASW
    # semaphore lanes, assigned to SWDGE instructions in emission order, to a
    # single queue, so emissions must cycle queues 0..3).
    gstart = np.zeros((NB, NBUCKETS), np.int64)
    bucket_len = np.zeros(NBUCKETS, np.int64)
    for k in range(NBUCKETS):
        pos = 0
        for b in range(NB):
            gstart[b, k] = pos
            pos += int(R[b, k])
        bucket_len[k] = pos
    nwin = int(max(-(-int(bucket_len[k]) // WSLOTS)
                   for k in range(NBUCKETS)))
    # per-bucket window sizes; rounds must all have a window for every
    # bucket, so a bucket that runs out gets a minimal 128-slot dummy
    windows = []
    for k in range(NBUCKETS):
        ws = []
        rem = int(bucket_len[k])
        for _ in range(nwin):
            w = min(WSLOTS, rem) if rem > 0 else P
            ws.append(w)
            rem -= w
        windows.append(ws)
    wcols_k = [int(sum(ws)) // 16 for ws in windows]     # idx cols per bucket
    bucket_ioff = np.zeros(NBUCKETS, np.int64)
    for k in range(1, NBUCKETS):
        bucket_ioff[k] = bucket_ioff[k - 1] + wcols_k[k - 1]
    sumW16 = int(sum(wcols_k))

    per_core = []
    for c in range(N_CORES):
        s_c, d_c, off = blocks[c]
        idx_cols = np.zeros((16, sumW16), np.int16)
        rseg_flat = np.full(sumT * P, -1.0, np.float32)
        for b in range(NB):
            for k in range(NBUCKETS):
                n = int(counts[c, b, k])
                if n == 0:
                    continue
                o = int(off[b * NBUCKETS + k])
                flat = np.zeros(int(R[b, k]), np.int64)
                flat[:n] = s_c[o:o + n] - k * BUCKET_SZ
                w0 = int(bucket_ioff[k]) + int(gstart[b, k]) // 16
                idx_cols[:, w0:w0 + int(R[b, k]) // 16] = flat.reshape(
                    -1, 16).T
                base = int(col_off[b, k]) * P
                rseg_flat[base:base + n] = (d_c[o:o + n] - b * P).astype(
                    np.float32)
        idx16 = np.tile(np.ascontiguousarray(idx_cols), (8, 1))
        rseg_t = np.ascontiguousarray(
            rseg_flat.reshape(sumT, P).T).astype(BF16)
        degc = np.zeros(NPAD, np.int32)
        degc[:NPC] = deg_full[c * NPC:(c + 1) * NPC]
        deg_t = np.ascontiguousarray(degc.reshape(NB, P).T)
        per_core.append({"idx16": idx16, "rseg": rseg_t, "deg": deg_t})

    sched = {
        "T": T, "R": R, "col_off": col_off, "gstart": gstart,
        "bucket_len": bucket_len, "bucket_ioff": bucket_ioff,
        "windows": windows, "wcols_k": wcols_k,
        "nwin": nwin, "sumT": sumT, "sumW16": sumW16,
        "nmm": nmm, "nmmax": nmmax,
    }
    return per_core, sched


def build_program(sched, n_nodes=N_NODES, nb=NB, npad=NPAD):
    """Trace + compile the SPMD Bass program for the given group schedule."""
    from contextlib import ExitStack

    from concourse import bacc, mybir, tile

    f32 = mybir.dt.float32
    bf16 = mybir.dt.bfloat16
    i32 = mybir.dt.int32
    i16 = mybir.dt.int16
    AL = mybir.AluOpType

    T = sched["T"]
    R = sched["R"]
    col_off = sched["col_off"]
    gstart = sched["gstart"]
    bucket_ioff = sched["bucket_ioff"]
    windows = sched["windows"]
    wcols_k = sched["wcols_k"]
    nwin = sched["nwin"]
    sumT = sched["sumT"]
    sumW16 = sched["sumW16"]
    nmm = sched["nmm"]
    nmmax = sched["nmmax"]

    nc = bacc.Bacc("TRN2", target_bir_lowering=False, debug=False,
                   num_devices=N_CORES, num_swdge_queues=NBUCKETS,
                   dynamic_dma_scratch_size=RING)
    h_ap = nc.dram_tensor("h16", [n_nodes, D], bf16, kind="ExternalInput").ap()
    hT_ap = nc.dram_tensor("hT16", [P, npad], bf16, kind="ExternalInput").ap()
    idx_ap = nc.dram_tensor("idx16", [P, sumW16], i16,
                            kind="ExternalInput").ap()
    rseg_ap = nc.dram_tensor("rseg", [P, sumT], bf16,
                             kind="ExternalInput").ap()
    deg_ap = nc.dram_tensor("deg", [P, nb], i32, kind="ExternalInput").ap()
    iota_ap = nc.dram_tensor("iotaw", [P, nmmax * P], bf16,
                             kind="ExternalInput").ap()
    ws_ap = nc.dram_tensor("W_self", [H, D, D], f32, kind="ExternalInput").ap()
    wn_ap = nc.dram_tensor("W_neigh", [H, D, D], f32,
                           kind="ExternalInput").ap()
    b_ap = nc.dram_tensor("b", [H, D], f32, kind="ExternalInput").ap()
    out_ap = nc.dram_tensor("out", [npad, D], bf16, kind="ExternalOutput").ap()

    bucket_aps = []
    for k in range(NBUCKETS):
        lo = k * BUCKET_SZ
        hi = min(n_nodes, lo + BUCKET_SZ)
        bucket_aps.append(h_ap[lo:hi, :])

    # window start slot per (bucket, window)
    wstart = []
    for k in range(NBUCKETS):
        acc, ss = 0, []
        for w in windows[k]:
            ss.append(acc)
            acc += w
        wstart.append(ss)

    with tile.TileContext(nc) as tc, ExitStack() as ctx:
        const = ctx.enter_context(tc.tile_pool(name="const", bufs=1))
        eps = [ctx.enter_context(tc.tile_pool(name=f"eg{k}", bufs=3))
               for k in range(NBUCKETS)]
        mp = ctx.enter_context(tc.tile_pool(name="msel", bufs=2))
        sp = ctx.enter_context(tc.tile_pool(name="small", bufs=2))
        hp = ctx.enter_context(tc.tile_pool(name="hts", bufs=2))
        pseg = ctx.enter_context(tc.tile_pool(name="pseg", bufs=2,
                                              space="PSUM"))
        pout = ctx.enter_context(tc.tile_pool(name="pout", bufs=2,
                                              space="PSUM"))
        pslf = ctx.enter_context(tc.tile_pool(name="pslf", bufs=2,
                                              space="PSUM"))

        # ---- prologue: first-window indices first so gathers start early ----
        idx_w0 = []
        idx_rest = []
        for k in range(NBUCKETS):
            c0 = windows[k][0] // 16
            t0 = const.tile([P, c0], i16, tag=f"idxw0_{k}")
            nc.sync.dma_start(t0[:], idx_ap[:, int(bucket_ioff[k]):
                                            int(bucket_ioff[k]) + c0])
            idx_w0.append(t0)
        for k in range(NBUCKETS):
            c0 = windows[k][0] // 16
            cr = wcols_k[k] - c0
            tr = const.tile([P, cr], i16, tag=f"idxrest_{k}")
            nc.sync.dma_start(tr[:], idx_ap[:, int(bucket_ioff[k]) + c0:
                                            int(bucket_ioff[k]) + wcols_k[k]])
            idx_rest.append(tr)

        iotaw = const.tile([P, nmmax * P], bf16, tag="iotaw")
        nc.sync.dma_start(iotaw[:], iota_ap)

        # head-averaged weights: wm = 0.25 * sum_h W[h], cast to bf16
        wmeans = []
        for name, ap in (("ws", ws_ap), ("wn", wn_ap)):
            heads = []
            for hh in range(H):
                t = const.tile([P, P], f32, tag=f"{name}h{hh}")
                nc.sync.dma_start(t[:], ap[hh])
                heads.append(t)
            s01 = const.tile([P, P], f32, tag=f"{name}s01")
            nc.vector.tensor_tensor(s01[:], heads[0][:], heads[1][:],
                                    op=AL.add)
            s23 = const.tile([P, P], f32, tag=f"{name}s23")
            nc.vector.tensor_tensor(s23[:], heads[2][:], heads[3][:],
                                    op=AL.add)
            s = const.tile([P, P], f32, tag=f"{name}sum")
            nc.vector.tensor_tensor(s[:], s01[:], s23[:], op=AL.add)
            wm = const.tile([P, P], bf16, tag=f"{name}m")
            nc.scalar.mul(wm[:], s[:], 1.0 / H)
            wmeans.append(wm)
        wsm, wnm = wmeans

        # bias matmul operands: q[h, m] = 1/H; pself += q.T @ b_sb
        b_sb = const.tile([H, P], f32, tag="bsb")
        nc.sync.dma_start(b_sb[:], b_ap)
        b16 = const.tile([H, P], bf16, tag="b16")
        nc.vector.tensor_copy(b16[:], b_sb[:])
        q16 = const.tile([H, P], bf16, tag="q16")
        nc.vector.memset(q16[:], 1.0 / H)

        # inverse degree: 1 / max(deg, 1)
        degsb = const.tile([P, nb], i32, tag="degsb")
        nc.sync.dma_start(degsb[:], deg_ap)
        degf = const.tile([P, nb], f32, tag="degf")
        nc.vector.tensor_copy(degf[:], degsb[:])
        nc.vector.tensor_scalar_max(degf[:], degf[:], 1.0)
        invd = const.tile([P, nb], f32, tag="invd")
        nc.vector.reciprocal(invd[:], degf[:])

        rseg_all = const.tile([P, sumT], bf16, tag="rseg_all")
        nc.sync.dma_start(rseg_all[:], rseg_ap)

        win_tiles = [{} for _ in range(NBUCKETS)]
        rounds = [0]
        prev_gather = [None]

        def issue_round():
            w = rounds[0]
            for k in range(NBUCKETS):
                W_w = windows[k][w]
                E = eps[k].tile([P, WSLOTS], bf16, tag=f"E{k}")
                if w == 0:
                    iap = idx_w0[k][:, :W_w // 16]
                else:
                    i0 = wstart[k][w] // 16 - windows[k][0] // 16
                    iap = idx_rest[k][:, i0:i0 + W_w // 16]
                g = nc.gpsimd.dma_gather(
                    E[:, :W_w].rearrange("p (c d) -> p c d", d=D),
                    bucket_aps[k],
                    iap,
                    W_w,
                    W_w,
                    D,
                    # single_packet coalesces a call's descriptors into ONE
                    # CME packet per engine; the HW packet ceiling is 64
                    # descs / 16KB, so >1024-idx calls must not coalesce
                    single_packet=False,
                    queue_num=k,
                )
                # chain gathers so the Tile scheduler keeps emission order
                # (its 8 DMASW sem lanes are assigned round-robin in schedule
                # order and each lane must stay on one SWDGE queue)
                if prev_gather[0] is not None:
                    from concourse.instruction_name_ordered_set import (
                        InstructionNameOrderedSet)
                    s = InstructionNameOrderedSet()
                    s.add(prev_gather[0])
                    g.ins.add_nosync_dependencies_from(s)
                prev_gather[0] = g.ins.name
                win_tiles[k][w] = E
                if w - 3 in win_tiles[k]:
                    del win_tiles[k][w - 3]
            rounds[0] = w + 1

        for _ in range(min(2, nwin)):
            issue_round()

        # ---- main loop over 128-node dst blocks ----
        for b in range(nb):
            # prefetch: ensure windows covering this block (+1 ahead) issued
            need = max((int(gstart[b, k]) + int(R[b, k]) - P) // WSLOTS
                       for k in range(NBUCKETS))
            while rounds[0] <= min(need + 1, nwin - 1):
                issue_round()

            nb_mm = int(nmm[b])
            c0 = int(col_off[b, 0])

            # wide msel: one is_equal over all of this block's sub-tiles
            msel = mp.tile([P, nmmax * P], bf16, tag="msel")
            nc.vector.tensor_tensor(
                out=msel[:, :nb_mm * P].rearrange("p (c d) -> p c d", d=P),
                in0=rseg_all[:, c0:c0 + nb_mm].to_broadcast([P, nb_mm, P]),
                in1=iotaw[:, :nb_mm * P].rearrange("p (c d) -> p c d", d=P),
                op=AL.is_equal,
            )

            # segment-sum, transposed: psT[feat, node] += E_t.T @ msel_t
            ps = pseg.tile([P, P], f32, tag="seg")
            i = 0
            for k in range(NBUCKETS):
                Tk = int(T[b, k])
                mo = int(col_off[b, k]) - c0
                for t in range(Tk):
                    slot = int(gstart[b, k]) + t * P
                    E = win_tiles[k][slot // WSLOTS]
                    off = slot % WSLOTS
                    nc.tensor.matmul(
                        ps[:],
                        lhsT=E[:, off:off + P],
                        rhs=msel[:, (mo + t) * P:(mo + t + 1) * P],
                        start=(i == 0),
                        stop=(i == nb_mm - 1),
                    )
                    i += 1

            # S^T to SBUF in bf16 (ACT engine)
            psb = sp.tile([P, P], bf16, tag="psb")
            nc.scalar.copy(psb[:], ps[:])

            # neighbor term: po[node, dout] = S.T.T @ wnm  (un-normalized)
            po = pout.tile([P, P], f32, tag="po")
            nc.tensor.matmul(po[:], lhsT=psb[:], rhs=wnm[:],
                             start=True, stop=True)

            # self term + bias: pself[node, dout] = h_blk @ wsm + 1/H sum b
            if b % HT_CHUNK == 0:
                hw = min(HT_CHUNK, nb - b) * P
                hTt4 = hp.tile([P, HT_CHUNK * P], bf16, tag="hT4")
                nc.sync.dma_start(hTt4[:, :hw],
                                  hT_ap[:, b * P:b * P + hw])
            pf = pslf.tile([P, P], f32, tag="pself")
            ho = (b % HT_CHUNK) * P
            nc.tensor.matmul(pf[:], lhsT=hTt4[:, ho:ho + P], rhs=wsm[:],
                             start=True, stop=False)
            nc.tensor.matmul(pf[:], lhsT=q16[:], rhs=b16[:],
                             start=False, stop=True)

            # pself to SBUF on ACT (PSUM has one DVE read port; keep the
            # fused DVE op below to a single PSUM operand)
            pfs = sp.tile([P, P], f32, tag="pfs")
            nc.scalar.copy(pfs[:], pf[:])

            # out = po * invd[node] + pself  (fused, bf16 out)
            ob = sp.tile([P, P], bf16, tag="ob")
            nc.vector.scalar_tensor_tensor(
                out=ob[:], in0=po[:], scalar=invd[:, b:b + 1], in1=pfs[:],
                op0=AL.mult, op1=AL.add,
            )
            nc.sync.dma_start(out_ap[b * P:(b + 1) * P, :], ob[:])

    nc.compile()
    return nc


_CACHE = {}


def kernel(h, src, dst, W_self, W_neigh, b):
    return run(h, src, dst, W_self, W_neigh, b)[0]


def run(h, src, dst, W_self, W_neigh, b, trace=False, **kw):
    from concourse.bass_utils import run_bass_kernel_spmd

    h = np.ascontiguousarray(np.asarray(h, dtype=np.float32))
    src = np.asarray(src, dtype=np.int32)
    dst = np.asarray(dst, dtype=np.int32)
    W_self = np.ascontiguousarray(np.asarray(W_self, dtype=np.float32))
    W_neigh = np.ascontiguousarray(np.asarray(W_neigh, dtype=np.float32))
    b = np.ascontiguousarray(np.asarray(b, dtype=np.float32))

    per_core, sched = _preprocess(src, dst)

    key = (tuple(sched["R"].ravel().tolist()),)
    if key not in _CACHE:
        _CACHE[key] = build_program(sched)
    nc = _CACHE[key]

    nmmax = sched["nmmax"]
    iotaw = np.ascontiguousarray(
        np.tile(np.arange(P, dtype=np.float32), (P, nmmax))).astype(BF16)
    h16 = h.astype(BF16)
    in_maps = []
    for c in range(N_CORES):
        hTc = np.zeros((P, NPAD), np.float32)
        hTc[:, :NPC] = h[c * NPC:(c + 1) * NPC].T
        in_maps.append({
            "h16": h16,
            "hT16": np.ascontiguousarray(hTc).astype(BF16),
            "idx16": per_core[c]["idx16"],
            "rseg": per_core[c]["rseg"],
            "deg": per_core[c]["deg"],
            "iotaw": iotaw,
            "W_self": W_self,
            "W_neigh": W_neigh,
            "b": b,
        })

    res = run_bass_kernel_spmd(nc, in_maps, core_ids=list(range(N_CORES)),
                               trace=trace, **kw)
    out = np.concatenate([res.results[c]["out"][:NPC].astype(np.float32)
                          for c in range(N_CORES)], axis=0)
    return out, res


# revision 12
# speedup vs baseline: 1.3676x; 1.2239x over previous
"""Multi-head SAGE layer (mean aggregation) as a Bass/Tile kernel on 8 trn2 cores.

Math: out = mean_h( h @ W_self[h] + segmean(h[src] by dst) @ W_neigh[h] + b[h] )
    = h @ mean_h(W_self) + segmean(h[src] by dst) @ mean_h(W_neigh) + mean_h(b)
(mean over heads commutes with the linear layers).

Sharding: nodes (and their incident edges, keyed by dst) are split across the
8 cores, 12500 nodes each; h is replicated per-core (bf16 copy) as the gather
table.  Edges are grouped by (src-range bucket, 128-node dst block), sorted
by src, and padded per group to R[b,k] = T[b,k]*128 slots (T = max count over
the 8 cores so the SPMD schedule is shared; pad slots gather row 0 of the
bucket and are masked out of the segment-sum by rseg = -1).  Each bucket's
slot stream is cut into 4096-slot gather windows (big windows amortize the
~1us fixed Q7 ucode cost per dma_gather call -- the SWDGE descriptor
generation on the GpSimd engine is strictly serialized and paces the whole
kernel); the last window per bucket is exactly the stream remainder, so no
tail slots exist at all.

The segment-sum runs on the tensor engine in bf16, directly in transposed
orientation: per 128-edge sub-tile, matmul(lhsT=E, rhs=msel) accumulates
S^T[feat, node] in PSUM, where msel[e, n] = (dst_local[e] == n) is built for
ALL of a block's sub-tiles with a single wide vector is_equal (stride-0
broadcast AP).  S^T feeds the W_neigh matmul without any transpose; the
1/deg normalization and the self-term addition are fused into one
scalar_tensor_tensor op: out = (po * invd) + pself, written in bf16.

All graph-structure preprocessing (edge partition/sort/padding, degree
counts, layout transposes, bf16 casts) happens on the host; all matrix math
on the features/weights happens on-device.
"""

import sys

import ml_dtypes
import numpy as np

for _p in ("/opt/trn_rl_repo",):
    if _p not in sys.path:
        sys.path.insert(0, _p)

BF16 = ml_dtypes.bfloat16

N_NODES = 100000
N_EDGES = 1600000
D = 128
H = 4
N_CORES = 8
P = 128
NPC = N_NODES // N_CORES          # nodes per core
NB = (NPC + P - 1) // P           # 128-node blocks per core
NPAD = NB * P                     # padded nodes per core
NBUCKETS = 4
BUCKET_SZ = -(-N_NODES // NBUCKETS)   # src rows per gather bucket (int16 range)
WSLOTS = 4096                     # gather-window slots per call
RING = 65536                      # dynamic_dma_scratch_size
HT_CHUNK = 4                      # dst blocks per hT load (bigger descriptors)
MCAP = 16                         # max msel sub-tiles built per is_equal phase


def _assign_nodes(src, dst):
    """Degree-balanced node -> (core, block, slot) assignment.

    The mean edge count per (128-dst-block, src-bucket) group sits just
    under the 512 = 4*128 matmul sub-tile boundary, so a balanced block
    composition makes nearly every group cost T=4 sub-tiles.  The excess is
    concentrated: the top 128*N_CORES highest-degree nodes go to one
    "sacrifice" block per core (block 0); the rest are packed greedily under
    a 512-per-bucket limit.  Full blocks are sorted by load so heavy blocks
    share a block index across cores (T = max over cores).

    Returns node_order [N_CORES, NPAD] int64, -1 for pad slots.
    """
    dv = np.zeros((N_NODES, NBUCKETS), np.int64)
    np.add.at(dv, (dst.astype(np.int64), src.astype(np.int64) // BUCKET_SZ), 1)
    tot = dv.sum(1)
    order = np.argsort(-tot, kind="stable")
    sac_nodes = order[:N_CORES * P]
    rest = order[N_CORES * P:]

    sac_loads = np.zeros((N_CORES, NBUCKETS), np.int64)
    sac_fill = np.zeros(N_CORES, np.int64)
    sac_members = [[] for _ in range(N_CORES)]
    for v in sac_nodes:
        sc = (sac_loads + dv[v]).max(1).astype(np.float64)
        sc[sac_fill >= P] = np.inf
        c = int(np.argmin(sc))
        sac_members[c].append(v)
        sac_loads[c] += dv[v]
        sac_fill[c] += 1

    nfull = NB - 2                      # full blocks per core (1..NB-2)
    ntail = NPC - P - nfull * P         # nodes in the core's last block
    nbins = N_CORES * (nfull + 1)
    cap = np.tile(np.r_[np.full(nfull, P), [ntail]], N_CORES)
    tail_lim = ((ntail * 16 // NBUCKETS) + P - 1) // P * P
    limv = np.tile(np.r_[np.full(nfull, 4 * P), [tail_lim]], N_CORES)
    lim = limv[:, None]
    loads = np.zeros((nbins, NBUCKETS), np.int64)
    fill = np.zeros(nbins, np.int64)
    members = [[] for _ in range(nbins)]
    BIG = 1 << 40
    for v in rest:
        d = dv[v]
        nl = loads + d
        over = np.maximum(nl - lim, 0).sum(1)
        score = over * 100000 + nl.max(1)
        score[fill >= cap] = BIG
        bi = int(np.argmin(score))
        members[bi].append(v)
        loads[bi] += d
        fill[bi] += 1

    node_order = np.full((N_CORES, NPAD), -1, np.int64)
    for c in range(N_CORES):
        node_order[c, :P] = sac_members[c]
        key = loads.reshape(N_CORES, nfull + 1, NBUCKETS)[c, :nfull].max(1)
        for bpos, bsrc in enumerate(np.argsort(-key, kind="stable")):
            mem = members[c * (nfull + 1) + int(bsrc)]
            node_order[c, (1 + bpos) * P:(1 + bpos) * P + len(mem)] = mem
        mem = members[c * (nfull + 1) + nfull]
        node_order[c, (NB - 1) * P:(NB - 1) * P + len(mem)] = mem
    return node_order


def _preprocess(src, dst):
    """Partition edges by dst owner core, group by (128-node dst block, src
    bucket), sort by src within each group, pad each group to R[b,k] slots.

    Returns (per_core, sched) where per_core[c]:
      idx16 [128, sumW16] int16  bucket-local gather indices, wrapped-16
                                 layout, one contiguous stream per bucket;
                                 pad = 0
      rseg  [128, sumT] bf16     dst id local to the block in [0,128),
                                 pad = -1; column order (b, k, t)
      deg   [128, NB]   int32    in-degree per node, column b = block b
      node_order [NPAD] int64    node id at each (block, slot), -1 = pad
    """
    src = src.astype(np.int64)
    dst = dst.astype(np.int64)
    node_order = _assign_nodes(src, dst)
    # node -> (core, local slot id in [0, NPAD))
    core_of = np.zeros(N_NODES, np.int64)
    lid_of = np.zeros(N_NODES, np.int64)
    for c in range(N_CORES):
        mask = node_order[c] >= 0
        ids = node_order[c][mask]
        core_of[ids] = c
        lid_of[ids] = np.nonzero(mask)[0]
    deg_full = np.bincount(dst, minlength=N_NODES).astype(np.int32)

    counts = np.zeros((N_CORES, NB, NBUCKETS), np.int64)
    blocks = []
    for c in range(N_CORES):
        m = core_of[dst] == c
        s_c = src[m]
        d_c = lid_of[dst[m]]
        blk = d_c >> 7
        bkt = s_c // BUCKET_SZ
        order = np.lexsort((s_c, bkt, blk))
        s_c, d_c, blk, bkt = s_c[order], d_c[order], blk[order], bkt[order]
        key = (blk * NBUCKETS + bkt)
        bc = np.bincount(key, minlength=NB * NBUCKETS)
        counts[c] = bc.reshape(NB, NBUCKETS)
        off = np.zeros(NB * NBUCKETS + 1, np.int64)
        np.cumsum(bc, out=off[1:])
        blocks.append((s_c, d_c, off))

    cmax = counts.max(axis=0)                            # [NB, NBUCKETS]
    T = (-(-np.maximum(1, cmax) // P)).astype(np.int64)  # matmul sub-tiles
    R = T * P                                            # gather slots

    # rseg/msel column offsets, (b, k, t) order (block-major)
    col_off = np.zeros((NB, NBUCKETS), np.int64)
    acc_t = 0
    for b in range(NB):
        for k in range(NBUCKETS):
            col_off[b, k] = acc_t
            acc_t += T[b, k]
    sumT = int(acc_t)
    nmm = T.sum(axis=1).astype(np.int64)                 # matmuls per block
    nmmax = int(nmm.max())

    # per-bucket slot streams: group (b,k) starts at gstart[b,k].  Streams
    # are cut into WSLOTS-sized gather windows; the last window per bucket is
    # the exact remainder (all windows share count nwin so calls can be
    # issued in strict queue round-robin -- Tile locks each of its 8 DMASW
    # semaphore lanes, assigned to SWDGE instructions in emission order, to a
    # single queue, so emissions must cycle queues 0..3).
    gstart = np.zeros((NB, NBUCKETS), np.int64)
    bucket_len = np.zeros(NBUCKETS, np.int64)
    for k in range(NBUCKETS):
        pos = 0
        for b in range(NB):
            gstart[b, k] = pos
            pos += int(R[b, k])
        bucket_len[k] = pos
    nwin = int(max(-(-int(bucket_len[k]) // WSLOTS)
                   for k in range(NBUCKETS)))
    # per-bucket window sizes; rounds must all have a window for every
    # bucket, so a bucket that runs out gets a minimal 128-slot dummy
    windows = []
    for k in range(NBUCKETS):
        ws = []
        rem = int(bucket_len[k])
        for _ in range(nwin):
            w = min(WSLOTS, rem) if rem > 0 else P
            ws.append(w)
            rem -= w
        windows.append(ws)
    wcols_k = [int(sum(ws)) // 16 for ws in windows]     # idx cols per bucket
    bucket_ioff = np.zeros(NBUCKETS, np.int64)
    for k in range(1, NBUCKETS):
        bucket_ioff[k] = bucket_ioff[k - 1] + wcols_k[k - 1]
    sumW16 = int(sum(wcols_k))

    per_core = []
    for c in range(N_CORES):
        s_c, d_c, off = blocks[c]
        idx_cols = np.zeros((16, sumW16), np.int16)
        rseg_flat = np.full(sumT * P, -1.0, np.float32)
        for b in range(NB):
            for k in range(NBUCKETS):
                n = int(counts[c, b, k])
                if n == 0:
                    continue
                o = int(off[b * NBUCKETS + k])
                flat = np.zeros(int(R[b, k]), np.int64)
                flat[:n] = s_c[o:o + n] - k * BUCKET_SZ
                w0 = int(bucket_ioff[k]) + int(gstart[b, k]) // 16
                idx_cols[:, w0:w0 + int(R[b, k]) // 16] = flat.reshape(
                    -1, 16).T
                base = int(col_off[b, k]) * P
                rseg_flat[base:base + n] = (d_c[o:o + n] - b * P).astype(
                    np.float32)
        idx16 = np.tile(np.ascontiguousarray(idx_cols), (8, 1))
        rseg_t = np.ascontiguousarray(
            rseg_flat.reshape(sumT, P).T).astype(BF16)
        no_c = node_order[c]
        degc = np.zeros(NPAD, np.int32)
        degc[no_c >= 0] = deg_full[no_c[no_c >= 0]]
        deg_t = np.ascontiguousarray(degc.reshape(NB, P).T)
        per_core.append({"idx16": idx16, "rseg": rseg_t, "deg": deg_t,
                         "node_order": no_c})

    sched = {
        "T": T, "R": R, "col_off": col_off, "gstart": gstart,
        "bucket_len": bucket_len, "bucket_ioff": bucket_ioff,
        "windows": windows, "wcols_k": wcols_k,
        "nwin": nwin, "sumT": sumT, "sumW16": sumW16,
        "nmm": nmm, "nmmax": nmmax,
    }
    return per_core, sched


def build_program(sched, n_nodes=N_NODES, nb=NB, npad=NPAD):
    """Trace + compile the SPMD Bass program for the given group schedule."""
    from contextlib import ExitStack

    from concourse import bacc, mybir, tile

    f32 = mybir.dt.float32
    bf16 = mybir.dt.bfloat16
    i32 = mybir.dt.int32
    i16 = mybir.dt.int16
    AL = mybir.AluOpType

    T = sched["T"]
    R = sched["R"]
    col_off = sched["col_off"]
    gstart = sched["gstart"]
    bucket_ioff = sched["bucket_ioff"]
    windows = sched["windows"]
    wcols_k = sched["wcols_k"]
    nwin = sched["nwin"]
    sumT = sched["sumT"]
    sumW16 = sched["sumW16"]
    nmm = sched["nmm"]
    nmmax = sched["nmmax"]

    nc = bacc.Bacc("TRN2", target_bir_lowering=False, debug=False,
                   num_devices=N_CORES, num_swdge_queues=NBUCKETS,
                   dynamic_dma_scratch_size=RING)
    h_ap = nc.dram_tensor("h16", [n_nodes, D], bf16, kind="ExternalInput").ap()
    hT_ap = nc.dram_tensor("hT16", [P, npad], bf16, kind="ExternalInput").ap()
    idx_ap = nc.dram_tensor("idx16", [P, sumW16], i16,
                            kind="ExternalInput").ap()
    rseg_ap = nc.dram_tensor("rseg", [P, sumT], bf16,
                             kind="ExternalInput").ap()
    deg_ap = nc.dram_tensor("deg", [P, nb], i32, kind="ExternalInput").ap()
    iota_ap = nc.dram_tensor("iotaw", [P, MCAP * P], bf16,
                             kind="ExternalInput").ap()
    ws_ap = nc.dram_tensor("W_self", [H, D, D], f32, kind="ExternalInput").ap()
    wn_ap = nc.dram_tensor("W_neigh", [H, D, D], f32,
                           kind="ExternalInput").ap()
    b_ap = nc.dram_tensor("b", [H, D], f32, kind="ExternalInput").ap()
    out_ap = nc.dram_tensor("out", [npad, D], bf16, kind="ExternalOutput").ap()

    bucket_aps = []
    for k in range(NBUCKETS):
        lo = k * BUCKET_SZ
        hi = min(n_nodes, lo + BUCKET_SZ)
        bucket_aps.append(h_ap[lo:hi, :])

    # window start slot per (bucket, window)
    wstart = []
    for k in range(NBUCKETS):
        acc, ss = 0, []
        for w in windows[k]:
            ss.append(acc)
            acc += w
        wstart.append(ss)

    with tile.TileContext(nc) as tc, ExitStack() as ctx:
        const = ctx.enter_context(tc.tile_pool(name="const", bufs=1))
        eps = [ctx.enter_context(tc.tile_pool(name=f"eg{k}", bufs=3))
               for k in range(NBUCKETS)]
        mp = ctx.enter_context(tc.tile_pool(name="msel", bufs=2))
        sp = ctx.enter_context(tc.tile_pool(name="small", bufs=2))
        hp = ctx.enter_context(tc.tile_pool(name="hts", bufs=2))
        pseg = ctx.enter_context(tc.tile_pool(name="pseg", bufs=2,
                                              space="PSUM"))
        pout = ctx.enter_context(tc.tile_pool(name="pout", bufs=2,
                                              space="PSUM"))
        pslf = ctx.enter_context(tc.tile_pool(name="pslf", bufs=2,
                                              space="PSUM"))

        # ---- prologue: first-window indices first so gathers start early ----
        idx_w0 = []
        idx_rest = []
        for k in range(NBUCKETS):
            c0 = windows[k][0] // 16
            t0 = const.tile([P, c0], i16, tag=f"idxw0_{k}")
            nc.sync.dma_start(t0[:], idx_ap[:, int(bucket_ioff[k]):
                                            int(bucket_ioff[k]) + c0])
            idx_w0.append(t0)
        for k in range(NBUCKETS):
            c0 = windows[k][0] // 16
            cr = wcols_k[k] - c0
            tr = const.tile([P, cr], i16, tag=f"idxrest_{k}")
            nc.sync.dma_start(tr[:], idx_ap[:, int(bucket_ioff[k]) + c0:
                                            int(bucket_ioff[k]) + wcols_k[k]])
            idx_rest.append(tr)

        iotaw = const.tile([P, MCAP * P], bf16, tag="iotaw")
        nc.sync.dma_start(iotaw[:], iota_ap)

        # head-averaged weights: wm = 0.25 * sum_h W[h], cast to bf16
        wmeans = []
        for name, ap in (("ws", ws_ap), ("wn", wn_ap)):
            heads = []
            for hh in range(H):
                t = const.tile([P, P], f32, tag=f"{name}h{hh}")
                nc.sync.dma_start(t[:], ap[hh])
                heads.append(t)
            s01 = const.tile([P, P], f32, tag=f"{name}s01")
            nc.vector.tensor_tensor(s01[:], heads[0][:], heads[1][:],
                                    op=AL.add)
            s23 = const.tile([P, P], f32, tag=f"{name}s23")
            nc.vector.tensor_tensor(s23[:], heads[2][:], heads[3][:],
                                    op=AL.add)
            s = const.tile([P, P], f32, tag=f"{name}sum")
            nc.vector.tensor_tensor(s[:], s01[:], s23[:], op=AL.add)
            wm = const.tile([P, P], bf16, tag=f"{name}m")
            nc.scalar.mul(wm[:], s[:], 1.0 / H)
            wmeans.append(wm)
        wsm, wnm = wmeans

        # bias matmul operands: q[h, m] = 1/H; pself += q.T @ b_sb
        b_sb = const.tile([H, P], f32, tag="bsb")
        nc.sync.dma_start(b_sb[:], b_ap)
        b16 = const.tile([H, P], bf16, tag="b16")
        nc.vector.tensor_copy(b16[:], b_sb[:])
        q16 = const.tile([H, P], bf16, tag="q16")
        nc.vector.memset(q16[:], 1.0 / H)

        # inverse degree: 1 / max(deg, 1)
        degsb = const.tile([P, nb], i32, tag="degsb")
        nc.sync.dma_start(degsb[:], deg_ap)
        degf = const.tile([P, nb], f32, tag="degf")
        nc.vector.tensor_copy(degf[:], degsb[:])
        nc.vector.tensor_scalar_max(degf[:], degf[:], 1.0)
        invd = const.tile([P, nb], f32, tag="invd")
        nc.vector.reciprocal(invd[:], degf[:])

        rseg_all = const.tile([P, sumT], bf16, tag="rseg_all")
        nc.sync.dma_start(rseg_all[:], rseg_ap)

        win_tiles = [{} for _ in range(NBUCKETS)]
        rounds = [0]
        prev_gather = [None]

        def issue_round():
            w = rounds[0]
            for k in range(NBUCKETS):
                W_w = windows[k][w]
                E = eps[k].tile([P, WSLOTS], bf16, tag=f"E{k}")
                if w == 0:
                    iap = idx_w0[k][:, :W_w // 16]
                else:
                    i0 = wstart[k][w] // 16 - windows[k][0] // 16
                    iap = idx_rest[k][:, i0:i0 + W_w // 16]
                g = nc.gpsimd.dma_gather(
                    E[:, :W_w].rearrange("p (c d) -> p c d", d=D),
                    bucket_aps[k],
                    iap,
                    W_w,
                    W_w,
                    D,
                    # single_packet coalesces a call's descriptors into ONE
                    # CME packet per engine; the HW packet ceiling is 64
                    # descs / 16KB, so >1024-idx calls must not coalesce
                    single_packet=False,
                    queue_num=k,
                )
                # chain gathers so the Tile scheduler keeps emission order
                # (its 8 DMASW sem lanes are assigned round-robin in schedule
                # order and each lane must stay on one SWDGE queue)
                if prev_gather[0] is not None:
                    from concourse.instruction_name_ordered_set import (
                        InstructionNameOrderedSet)
                    s = InstructionNameOrderedSet()
                    s.add(prev_gather[0])
                    g.ins.add_nosync_dependencies_from(s)
                prev_gather[0] = g.ins.name
                win_tiles[k][w] = E
                if w - 3 in win_tiles[k]:
                    del win_tiles[k][w - 3]
            rounds[0] = w + 1

        for _ in range(min(2, nwin)):
            issue_round()

        # ---- main loop over 128-node dst blocks ----
        for b in range(nb):
            # prefetch: ensure windows covering this block (+1 ahead) issued
            need = max((int(gstart[b, k]) + int(R[b, k]) - P) // WSLOTS
                       for k in range(NBUCKETS))
            while rounds[0] <= min(need + 1, nwin - 1):
                issue_round()

            nb_mm = int(nmm[b])
            c0 = int(col_off[b, 0])

            # flat (bucket, sub-tile) list in rseg column order
            subs = []
            for k in range(NBUCKETS):
                for t in range(int(T[b, k])):
                    subs.append((k, int(gstart[b, k]) + t * P))

            # segment-sum, transposed: psT[feat, node] += E_t.T @ msel_t.
            # msel (one is_equal over many sub-tiles) is built in phases of
            # at most MCAP sub-tiles to bound the tile size; PSUM
            # accumulation spans the phases.
            ps = pseg.tile([P, P], f32, tag="seg")
            for g0 in range(0, nb_mm, MCAP):
                g1 = min(nb_mm, g0 + MCAP)
                msel = mp.tile([P, MCAP * P], bf16, tag="msel")
                nc.vector.tensor_tensor(
                    out=msel[:, :(g1 - g0) * P].rearrange(
                        "p (c d) -> p c d", d=P),
                    in0=rseg_all[:, c0 + g0:c0 + g1].to_broadcast(
                        [P, g1 - g0, P]),
                    in1=iotaw[:, :(g1 - g0) * P].rearrange(
                        "p (c d) -> p c d", d=P),
                    op=AL.is_equal,
                )
                for g in range(g0, g1):
                    k, slot = subs[g]
                    E = win_tiles[k][slot // WSLOTS]
                    off = slot % WSLOTS
                    nc.tensor.matmul(
                        ps[:],
                        lhsT=E[:, off:off + P],
                        rhs=msel[:, (g - g0) * P:(g - g0 + 1) * P],
                        start=(g == 0),
                        stop=(g == nb_mm - 1),
                    )

            # S^T to SBUF in bf16 (ACT engine)
            psb = sp.tile([P, P], bf16, tag="psb")
            nc.scalar.copy(psb[:], ps[:])

            # neighbor term: po[node, dout] = S.T.T @ wnm  (un-normalized)
            po = pout.tile([P, P], f32, tag="po")
            nc.tensor.matmul(po[:], lhsT=psb[:], rhs=wnm[:],
                             start=True, stop=True)

            # self term + bias: pself[node, dout] = h_blk @ wsm + 1/H sum b
            if b % HT_CHUNK == 0:
                hw = min(HT_CHUNK, nb - b) * P
                hTt4 = hp.tile([P, HT_CHUNK * P], bf16, tag="hT4")
                nc.sync.dma_start(hTt4[:, :hw],
                                  hT_ap[:, b * P:b * P + hw])
            pf = pslf.tile([P, P], f32, tag="pself")
            ho = (b % HT_CHUNK) * P
            nc.tensor.matmul(pf[:], lhsT=hTt4[:, ho:ho + P], rhs=wsm[:],
                             start=True, stop=False)
            nc.tensor.matmul(pf[:], lhsT=q16[:], rhs=b16[:],
                             start=False, stop=True)

            # pself to SBUF on ACT (PSUM has one DVE read port; keep the
            # fused DVE op below to a single PSUM operand)
            pfs = sp.tile([P, P], f32, tag="pfs")
            nc.scalar.copy(pfs[:], pf[:])

            # out = po * invd[node] + pself  (fused, bf16 out)
            ob = sp.tile([P, P], bf16, tag="ob")
            nc.vector.scalar_tensor_tensor(
                out=ob[:], in0=po[:], scalar=invd[:, b:b + 1], in1=pfs[:],
                op0=AL.mult, op1=AL.add,
            )
            nc.sync.dma_start(out_ap[b * P:(b + 1) * P, :], ob[:])

    nc.compile()
    return nc


_CACHE = {}


def kernel(h, src, dst, W_self, W_neigh, b):
    return run(h, src, dst, W_self, W_neigh, b)[0]


def run(h, src, dst, W_self, W_neigh, b, trace=False, **kw):
    from concourse.bass_utils import run_bass_kernel_spmd

    h = np.ascontiguousarray(np.asarray(h, dtype=np.float32))
    src = np.asarray(src, dtype=np.int32)
    dst = np.asarray(dst, dtype=np.int32)
    W_self = np.ascontiguousarray(np.asarray(W_self, dtype=np.float32))
    W_neigh = np.ascontiguousarray(np.asarray(W_neigh, dtype=np.float32))
    b = np.ascontiguousarray(np.asarray(b, dtype=np.float32))

    per_core, sched = _preprocess(src, dst)

    key = (tuple(sched["R"].ravel().tolist()),)
    if key not in _CACHE:
        _CACHE[key] = build_program(sched)
    nc = _CACHE[key]

    iotaw = np.ascontiguousarray(
        np.tile(np.arange(P, dtype=np.float32), (P, MCAP))).astype(BF16)
    h16 = h.astype(BF16)
    in_maps = []
    for c in range(N_CORES):
        no_c = per_core[c]["node_order"]
        hTc = np.zeros((P, NPAD), np.float32)
        hTc[:, no_c >= 0] = h[no_c[no_c >= 0]].T
        in_maps.append({
            "h16": h16,
            "hT16": np.ascontiguousarray(hTc).astype(BF16),
            "idx16": per_core[c]["idx16"],
            "rseg": per_core[c]["rseg"],
            "deg": per_core[c]["deg"],
            "iotaw": iotaw,
            "W_self": W_self,
            "W_neigh": W_neigh,
            "b": b,
        })

    res = run_bass_kernel_spmd(nc, in_maps, core_ids=list(range(N_CORES)),
                               trace=trace, **kw)
    out = np.zeros((N_NODES, D), np.float32)
    for c in range(N_CORES):
        no_c = per_core[c]["node_order"]
        rows = np.asarray(res.results[c]["out"]).astype(np.float32)
        out[no_c[no_c >= 0]] = rows[no_c >= 0]
    return out, res
